# revision 29
# baseline (speedup 1.0000x reference)
"""Trainium2 Bass kernel for nn_Attention_17119739642103.

Math: the reference computes
    q = x @ Wq.T + bq ; k = x @ Wk.T + bk            # [B,N,O]
    ksum = k.sum(1)                                  # [B,O]
    s = (q @ ksum) * scale                           # [B,N]
    w = softmax(s, -1)                               # [B,N]
    agg = w @ x ; avgw = w.mean(0)

Shift-invariance of softmax drops the bq.ksum constant, so
    s[b,n] = x[b,n,:] . v[b]  (up to a per-b constant), where
    v[b]   = scale * (Wq.T @ Wk) @ xsum[b] + scale * N * (Wq.T @ bk)
    xsum[b]= x[b].sum(0)
The [256,256] weight algebra is folded on the host; the device does:
  S1: stream the batch into SBUF (fully cached, 16 MiB) + xsum via PE
  S2: scores via DVE fused multiply-reduce against broadcast v
  S3: softmax on-chip, then weighted-sum via PE (w as stationary operand)
Sharding: B=16 batches, 2 per core, data-parallel across 8 cores.
"""

import math
import sys

import numpy as np

_TRN_REPO = "/opt/trn_rl_repo"
if _TRN_REPO not in sys.path:
    sys.path.insert(0, _TRN_REPO)

B, N, D = 16, 16384, 256
NCORES = 8
BPC = B // NCORES  # batches per core
P = 128            # SBUF partitions
NT = N // P        # 128 row-blocks per batch; block t holds rows n = 128*p + t
SJ = 8             # row-blocks per super-tile (one DMA)
NST = NT // SJ     # 16 super-tiles per batch
SCALE = 1.0 / math.sqrt(D)

_compiled = {}


def _build_nc():
    from contextlib import ExitStack

    import concourse.bacc as bacc
    import concourse.tile as tile
    from concourse import bass_isa, mybir

    f32 = mybir.dt.float32
    mult = mybir.AluOpType.mult
    addop = mybir.AluOpType.add
    maxop = mybir.AluOpType.max

    nc = bacc.Bacc("TRN2", target_bir_lowering=False, debug=False)
    x_d = nc.dram_tensor("x_sh", [BPC, N, D], f32, kind="ExternalInput").ap()
    mt_d = nc.dram_tensor("mt", [2, P, D], f32, kind="ExternalInput").ap()
    us_d = nc.dram_tensor("us", [1, D], f32, kind="ExternalInput").ap()
    agg_d = nc.dram_tensor("agg", [BPC, D], f32, kind="ExternalOutput").ap()
    wsum_d = nc.dram_tensor("wsum", [N], f32, kind="ExternalOutput").ap()

    with ExitStack() as ctx:
        tc = ctx.enter_context(tile.TileContext(nc))
        xpool = ctx.enter_context(tc.tile_pool(name="xpool", bufs=19))
        singles = ctx.enter_context(tc.tile_pool(name="singles", bufs=1))
        small = ctx.enter_context(tc.tile_pool(name="small", bufs=2))
        psum = ctx.enter_context(tc.tile_pool(name="psum", bufs=2, space="PSUM"))

        mt_sb = singles.tile([P, 2, D], f32)
        nc.sync.dma_start(out=mt_sb, in_=mt_d.rearrange("c p d -> p c d"))
        us_sb = singles.tile([1, D], f32)
        nc.sync.dma_start(out=us_sb, in_=us_d)
        ones_sb = singles.tile([P, 1], f32)
        nc.vector.memset(ones_sb, 1.0)
        avgw_sb = singles.tile([P, NT], f32)

        def make_v(xsum_sb):
            # row -> columns via K=1 outer-product matmuls, then
            # v = scale*M@xsum + scale*N*Wq.T@bk as
            # v[1,d] = sum_c xsumT[:,c].T @ mt[c]  (+ us)
            xsT0_ps = psum.tile([P, 1], f32, name="xsT0_ps", bufs=1)
            xsT1_ps = psum.tile([P, 1], f32, name="xsT1_ps", bufs=1)
            nc.tensor.matmul(xsT0_ps, xsum_sb[0:1, 0:P], ones_sb[0:1, 0:1], start=True, stop=True)
            nc.tensor.matmul(xsT1_ps, xsum_sb[0:1, P : 2 * P], ones_sb[0:1, 0:1], start=True, stop=True)
            xsT_sb = small.tile([P, 2], f32, name="xsT_sb")
            nc.vector.tensor_copy(out=xsT_sb[:, 0:1], in_=xsT0_ps)
            nc.vector.tensor_copy(out=xsT_sb[:, 1:2], in_=xsT1_ps)
            v_ps = psum.tile([1, D], f32, name="v_ps", bufs=1)
            nc.tensor.matmul(v_ps, xsT_sb[:, 0:1], mt_sb[:, 0, :], start=True, stop=False)
            nc.tensor.matmul(v_ps, xsT_sb[:, 1:2], mt_sb[:, 1, :], start=False, stop=True)
            v_sb = small.tile([1, D], f32, name="v_sb")
            nc.vector.tensor_add(out=v_sb, in0=v_ps, in1=us_sb)
            v_rep = small.tile([P, D], f32, name="v_rep")
            nc.gpsimd.partition_broadcast(v_rep, v_sb)
            return v_rep

        prestream_v = [None]
        for b in range(BPC):
            # n = 128*p + 8*st + j  ->  [st, p, (j d)] tiles, 1 MiB per DMA,
            # 8 KiB contiguous per partition.
            xr = x_d[b].rearrange("(p st j) d -> st p (j d)", p=P, st=NST, j=SJ)
            xt = []
            # xsum2[1, 1024] accumulates [sum of even blocks | odd blocks]
            # (ones stationary, x moving at the fp32 max free dim of 512).
            nmm = SJ * D // 512  # 512-wide matmuls per super-tile
            if b == 0:
                xsum_ps = psum.tile([1, 2 * D], f32, name="xsum_ps", bufs=1)
            # PE sums the early supertiles (their data lands first); the DVE
            # (idle during S1) takes the late tail so the v hand-off is not
            # serialized behind either engine.
            NPE_X = 12
            if b == 0:
                acc_x = small.tile([P, SJ * D], f32, name="acc_x", bufs=1)
            for st in range(NST):
                x_t = xpool.tile([P, SJ * D], f32, name="x_t")
                nc.sync.dma_start(out=x_t, in_=xr[st])
                xt.append(x_t)
                if b > 0:
                    continue  # xsum+v already computed by the pre-stream pass
                if st < NPE_X:
                    for j2 in range(nmm):
                        nc.tensor.matmul(
                            xsum_ps, ones_sb, x_t[:, j2 * 512 : (j2 + 1) * 512],
                            start=(st == 0 and j2 == 0),
                            stop=False,
                        )
                elif st == NPE_X:
                    nc.vector.tensor_copy(out=acc_x, in_=x_t)
                else:
                    nc.vector.tensor_add(out=acc_x, in0=acc_x, in1=x_t)
            if b == 0:
                # partition-reduce the DVE accumulator straight into the
                # main xsum group (same even/odd [1,512] layout) - no
                # serial DVE fold chain on the critical path
                for j2 in range(nmm):
                    nc.tensor.matmul(
                        xsum_ps, ones_sb, acc_x[:, j2 * 512 : (j2 + 1) * 512],
                        start=False, stop=(j2 == nmm - 1),
                    )

                # xsum[1,256] = even-block sums + odd-block sums
                # (DVE can read only one PSUM operand per op: copy, then add)
                xsum_sb = small.tile([1, D], f32, name="xsum_sb")
                nc.vector.tensor_copy(out=xsum_sb, in_=xsum_ps[0:1, 0:D])
                nc.vector.tensor_add(
                    out=xsum_sb, in0=xsum_sb, in1=xsum_ps[0:1, D : 2 * D]
                )
                v_rep = make_v(xsum_sb)
            else:
                v_rep = prestream_v[0]

            if b == 0 and BPC > 1:
                # pre-stream batch 1 for its xsum while batch 0 scores run:
                # PE and the DMA engines are otherwise idle here, and this
                # removes the xsum -> v dependency from batch 1's critical
                # path (costs one extra HBM read of batch 1).
                xr1 = x_d[1].rearrange(
                    "(p st j) d -> st p (j d)", p=P, st=NST, j=SJ
                )
                xsum1_ps = psum.tile([1, 2 * D], f32, name="xsum1_ps", bufs=1)
                for st in range(NST):
                    xs_t = xpool.tile([P, SJ * D], f32, name="xs_t", bufs=3)
                    nc.sync.dma_start(out=xs_t, in_=xr1[st])
                    for j2 in range(nmm):
                        nc.tensor.matmul(
                            xsum1_ps, ones_sb, xs_t[:, j2 * 512 : (j2 + 1) * 512],
                            start=(st == 0 and j2 == 0),
                            stop=(st == NST - 1 and j2 == nmm - 1),
                        )
                xsum1_half = small.tile([1, D], f32, name="xsum_half")
                nc.vector.tensor_copy(out=xsum1_half, in_=xsum1_ps[0:1, 0:D])
                nc.vector.tensor_add(
                    out=xsum1_half, in0=xsum1_half, in1=xsum1_ps[0:1, D : 2 * D]
                )
                prestream_v[0] = make_v(xsum1_half)

            # S2: s[p,t] = x_blk[p,:] . v  via fused multiply+row-reduce
            # (custom DVE op: out = (in0*1+0)*v_rep, accum_out = rowsum)
            s_sb = small.tile([P, NT], f32, name="s_sb")
            warm_ps = psum.tile([1, D], f32, name="warm_ps", bufs=1)
            for st in range(NST):
                for j in range(SJ):
                    t = st * SJ + j
                    # in place: the tile becomes prod = x*v; agg is
                    # recovered as (sum_n w*prod)/v at the end. Frees no
                    # extra SBUF and lets the next batch's DMA chase the
                    # score sweep (x's last reader would otherwise be S3).
                    blk = xt[st][:, j * D : (j + 1) * D]
                    nc.vector.affine_mul_reduce(
                        out=blk,
                        accum_out=s_sb[:, t : t + 1],
                        in0=blk,
                        in1=v_rep,
                        scale=1.0,
                        bias=0.0,
                    )
                    if t % 6 == 5:
                        # keep-warm trickle: PE re-throttles to 1.2 GHz after
                        # ~3.4us idle; a junk matmul every ~6 score blocks
                        # (input-dependent on the score just computed, so it
                        # interleaves in time) keeps it at 2.4 GHz.
                        nc.tensor.matmul(warm_ps, ones_sb, blk, start=True, stop=True)

            # softmax over all 16384 scores of the batch
            m_p = small.tile([P, 1], f32, name="m_p")
            nc.vector.tensor_reduce(m_p, s_sb, axis=mybir.AxisListType.X, op=maxop)
            m_rep = small.tile([P, 1], f32, name="m_rep")
            nc.gpsimd.partition_all_reduce(
                m_rep, m_p, channels=P, reduce_op=bass_isa.ReduceOp.max
            )
            m_neg = small.tile([P, 1], f32, name="m_neg")
            nc.vector.tensor_scalar_mul(m_neg, m_rep, -1.0)
            e_sb = small.tile([P, NT], f32, name="e_sb")
            l_p = small.tile([P, 1], f32, name="l_p")
            nc.scalar.activation(
                out=e_sb,
                in_=s_sb,
                func=mybir.ActivationFunctionType.Exp,
                bias=m_neg,
                scale=1.0,
                accum_out=l_p,
            )
            l_rep = small.tile([P, 1], f32, name="l_rep")
            nc.gpsimd.partition_all_reduce(
                l_rep, l_p, channels=P, reduce_op=bass_isa.ReduceOp.add
            )
            linv = small.tile([P, 1], f32, name="linv")
            nc.vector.reciprocal(linv, l_rep)
            w_sb = small.tile([P, NT], f32, name="w_sb")
            nc.vector.tensor_scalar_mul(w_sb, e_sb, linv)
            if b == 0:
                nc.vector.tensor_copy(out=avgw_sb, in_=w_sb)
            else:
                nc.vector.tensor_add(out=avgw_sb, in0=avgw_sb, in1=w_sb)

            # S3: agg[1,d] = sum_t w[:,t].T @ x_blk, split across engines:
            # supertiles 0..7 as PE matmuls (w stationary), supertiles 8..15
            # as a DVE chain acc += x_blk * w[:,t] (per-partition scalar),
            # partition-reduced on PE at the end.
            agg_ps = psum.tile([1, D], f32, name="agg_ps", bufs=1)
            accA = small.tile([P, D], f32, name="accA", bufs=1)
            accB = small.tile([P, D], f32, name="accB", bufs=1)
            nc.vector.memset(accA, 0.0)
            nc.vector.memset(accB, 0.0)
            # batch 0 biases S3 toward PE so the DVE frees up sooner for
            # batch 1's scores (already runnable at that point); batch 1 is
            # terminal, so it balances the two engines evenly.
            s3_pe = 12 if (b == 0 and BPC > 1) else NST // 2
            for st in range(s3_pe):
                for j in range(SJ):
                    t = st * SJ + j
                    nc.tensor.matmul(
                        agg_ps,
                        w_sb[:, t : t + 1],
                        xt[st][:, j * D : (j + 1) * D],
                        start=(t == 0),
                        stop=False,
                    )
            # two independent in-place chains so consecutive DVE ops never
            # wait on each other
            for st in range(s3_pe, NST):
                for j in range(SJ):
                    t = st * SJ + j
                    acc = accA if j % 2 == 0 else accB
                    nc.vector.scalar_tensor_tensor(
                        out=acc,
                        in0=xt[st][:, j * D : (j + 1) * D],
                        scalar=w_sb[:, t : t + 1],
                        in1=acc,
                        op0=mult,
                        op1=addop,
                    )
            # fold the DVE accumulators into agg_ps (partition reduce)
            nc.tensor.matmul(agg_ps, ones_sb, accA, start=False, stop=False)
            nc.tensor.matmul(agg_ps, ones_sb, accB, start=False, stop=True)
            vinv = small.tile([1, D], f32, name="vinv")
            nc.vector.reciprocal(out=vinv, in_=v_rep[0:1, :])
            agg_sb = small.tile([1, D], f32, name="agg_sb")
            nc.vector.tensor_mul(out=agg_sb, in0=agg_ps, in1=vinv)
            nc.sync.dma_start(out=agg_d[b : b + 1, :], in_=agg_sb)

        nc.sync.dma_start(out=wsum_d.rearrange("(p t) -> p t", p=P), in_=avgw_sb)

    nc.compile()
    return nc


def _get_nc():
    if "nc" not in _compiled:
        _compiled["nc"] = _build_nc()
    return _compiled["nc"]


def _host_prep(inputs):
    x = np.ascontiguousarray(np.asarray(inputs["x"], dtype=np.float32))
    Wq = np.asarray(inputs["Wq"], dtype=np.float64)
    Wk = np.asarray(inputs["Wk"], dtype=np.float64)
    bk = np.asarray(inputs["bk"], dtype=np.float64)
    # M = Wq.T @ Wk ; device needs MT[c,i,d] = scale*M[d, 128c+i] = scale*(M.T)[128c+i, d]
    mt = (SCALE * (Wk.T @ Wq)).reshape(2, P, D).astype(np.float32)
    us = (SCALE * N * (Wq.T @ bk)).reshape(1, D).astype(np.float32)
    return x, np.ascontiguousarray(mt), np.ascontiguousarray(us)


def _run(inputs, **spmd_kwargs):
    from concourse.bass_utils import run_bass_kernel_spmd

    x, mt, us = _host_prep(inputs)
    nc = _get_nc()
    xs = x.reshape(NCORES, BPC, N, D)
    in_maps = [{"x_sh": xs[c], "mt": mt, "us": us} for c in range(NCORES)]
    res = run_bass_kernel_spmd(nc, in_maps, core_ids=list(range(NCORES)), **spmd_kwargs)
    agg = np.concatenate([res.results[c]["agg"] for c in range(NCORES)], axis=0)
    wsum = np.zeros(N, dtype=np.float64)
    for c in range(NCORES):
        wsum += res.results[c]["wsum"].astype(np.float64)
    avgw = (wsum / B).astype(np.float32)
    return (agg.astype(np.float32), avgw), res


def kernel(**inputs):
    out, _ = _run(inputs)
    return out


# revision 30
# speedup vs baseline: 1.0148x; 1.0148x over previous
"""Trainium2 Bass kernel for nn_Attention_17119739642103.

Math: the reference computes
    q = x @ Wq.T + bq ; k = x @ Wk.T + bk            # [B,N,O]
    ksum = k.sum(1)                                  # [B,O]
    s = (q @ ksum) * scale                           # [B,N]
    w = softmax(s, -1)                               # [B,N]
    agg = w @ x ; avgw = w.mean(0)

Shift-invariance of softmax drops the bq.ksum constant, so
    s[b,n] = x[b,n,:] . v[b]  (up to a per-b constant), where
    v[b]   = scale * (Wq.T @ Wk) @ xsum[b] + scale * N * (Wq.T @ bk)
    xsum[b]= x[b].sum(0)
The [256,256] weight algebra is folded on the host; the device does:
  S1: stream the batch into SBUF (fully cached, 16 MiB) + xsum via PE
  S2: scores via DVE fused multiply-reduce against broadcast v
  S3: softmax on-chip, then weighted-sum via PE (w as stationary operand)
Sharding: B=16 batches, 2 per core, data-parallel across 8 cores.
"""

import math
import sys

import numpy as np

_TRN_REPO = "/opt/trn_rl_repo"
if _TRN_REPO not in sys.path:
    sys.path.insert(0, _TRN_REPO)

B, N, D = 16, 16384, 256
NCORES = 8
BPC = B // NCORES  # batches per core
P = 128            # SBUF partitions
NT = N // P        # 128 row-blocks per batch; block t holds rows n = 128*p + t
SJ = 8             # row-blocks per super-tile (one DMA)
NST = NT // SJ     # 16 super-tiles per batch
SCALE = 1.0 / math.sqrt(D)

_compiled = {}


def _build_nc():
    from contextlib import ExitStack

    import concourse.bacc as bacc
    import concourse.tile as tile
    from concourse import bass_isa, mybir

    f32 = mybir.dt.float32
    mult = mybir.AluOpType.mult
    addop = mybir.AluOpType.add
    maxop = mybir.AluOpType.max

    nc = bacc.Bacc("TRN2", target_bir_lowering=False, debug=False)
    x_d = nc.dram_tensor("x_sh", [BPC, N, D], f32, kind="ExternalInput").ap()
    mt_d = nc.dram_tensor("mt", [2, P, D], f32, kind="ExternalInput").ap()
    us_d = nc.dram_tensor("us", [1, D], f32, kind="ExternalInput").ap()
    agg_d = nc.dram_tensor("agg", [BPC, D], f32, kind="ExternalOutput").ap()
    wsum_d = nc.dram_tensor("wsum", [N], f32, kind="ExternalOutput").ap()

    with ExitStack() as ctx:
        tc = ctx.enter_context(tile.TileContext(nc))
        xpool = ctx.enter_context(tc.tile_pool(name="xpool", bufs=19))
        singles = ctx.enter_context(tc.tile_pool(name="singles", bufs=1))
        small = ctx.enter_context(tc.tile_pool(name="small", bufs=2))
        psum = ctx.enter_context(tc.tile_pool(name="psum", bufs=2, space="PSUM"))

        mt_sb = singles.tile([P, 2, D], f32)
        nc.sync.dma_start(out=mt_sb, in_=mt_d.rearrange("c p d -> p c d"))
        us_sb = singles.tile([1, D], f32)
        nc.sync.dma_start(out=us_sb, in_=us_d)
        ones_sb = singles.tile([P, 1], f32)
        nc.vector.memset(ones_sb, 1.0)
        avgw_sb = singles.tile([P, NT], f32)

        def make_v(xsum_sb):
            # row -> columns via K=1 outer-product matmuls, then
            # v = scale*M@xsum + scale*N*Wq.T@bk as
            # v[1,d] = sum_c xsumT[:,c].T @ mt[c]  (+ us)
            xsT0_ps = psum.tile([P, 1], f32, name="xsT0_ps", bufs=1)
            xsT1_ps = psum.tile([P, 1], f32, name="xsT1_ps", bufs=1)
            nc.tensor.matmul(xsT0_ps, xsum_sb[0:1, 0:P], ones_sb[0:1, 0:1], start=True, stop=True)
            nc.tensor.matmul(xsT1_ps, xsum_sb[0:1, P : 2 * P], ones_sb[0:1, 0:1], start=True, stop=True)
            xsT_sb = small.tile([P, 2], f32, name="xsT_sb")
            nc.vector.tensor_copy(out=xsT_sb[:, 0:1], in_=xsT0_ps)
            nc.vector.tensor_copy(out=xsT_sb[:, 1:2], in_=xsT1_ps)
            v_ps = psum.tile([1, D], f32, name="v_ps", bufs=1)
            nc.tensor.matmul(v_ps, xsT_sb[:, 0:1], mt_sb[:, 0, :], start=True, stop=False)
            nc.tensor.matmul(v_ps, xsT_sb[:, 1:2], mt_sb[:, 1, :], start=False, stop=True)
            v_sb = small.tile([1, D], f32, name="v_sb")
            nc.vector.tensor_add(out=v_sb, in0=v_ps, in1=us_sb)
            v_rep = small.tile([P, D], f32, name="v_rep")
            nc.gpsimd.partition_broadcast(v_rep, v_sb)
            return v_rep

        prestream_v = [None]
        for b in range(BPC):
            # n = 128*p + 8*st + j  ->  [st, p, (j d)] tiles, 1 MiB per DMA,
            # 8 KiB contiguous per partition.
            xr = x_d[b].rearrange("(p st j) d -> st p (j d)", p=P, st=NST, j=SJ)
            xt = []
            # xsum2[1, 1024] accumulates [sum of even blocks | odd blocks]
            # (ones stationary, x moving at the fp32 max free dim of 512).
            nmm = SJ * D // 512  # 512-wide matmuls per super-tile
            if b == 0:
                xsum_ps = psum.tile([1, 2 * D], f32, name="xsum_ps", bufs=1)
            # PE sums the early supertiles (their data lands first); the DVE
            # (idle during S1) takes the late tail so the v hand-off is not
            # serialized behind either engine.
            NPE_X = 12
            if b == 0:
                acc_x = small.tile([P, SJ * D], f32, name="acc_x", bufs=1)
            for st in range(NST):
                x_t = xpool.tile([P, SJ * D], f32, name="x_t")
                nc.sync.dma_start(out=x_t, in_=xr[st])
                xt.append(x_t)
                if b > 0:
                    continue  # xsum+v already computed by the pre-stream pass
                if st < NPE_X:
                    for j2 in range(nmm):
                        nc.tensor.matmul(
                            xsum_ps, ones_sb, x_t[:, j2 * 512 : (j2 + 1) * 512],
                            start=(st == 0 and j2 == 0),
                            stop=False,
                        )
                elif st == NPE_X:
                    nc.vector.tensor_copy(out=acc_x, in_=x_t)
                else:
                    nc.vector.tensor_add(out=acc_x, in0=acc_x, in1=x_t)
            if b == 0:
                # partition-reduce the DVE accumulator straight into the
                # main xsum group (same even/odd [1,512] layout) - no
                # serial DVE fold chain on the critical path
                for j2 in range(nmm):
                    nc.tensor.matmul(
                        xsum_ps, ones_sb, acc_x[:, j2 * 512 : (j2 + 1) * 512],
                        start=False, stop=(j2 == nmm - 1),
                    )

                # xsum[1,256] = even-block sums + odd-block sums
                # (DVE can read only one PSUM operand per op: copy, then add)
                xsum_sb = small.tile([1, D], f32, name="xsum_sb")
                nc.vector.tensor_copy(out=xsum_sb, in_=xsum_ps[0:1, 0:D])
                nc.vector.tensor_add(
                    out=xsum_sb, in0=xsum_sb, in1=xsum_ps[0:1, D : 2 * D]
                )
                v_rep = make_v(xsum_sb)
            else:
                v_rep = prestream_v[0]

            if b == 0 and BPC > 1:
                # pre-stream batch 1 for its xsum while batch 0 scores run:
                # PE and the DMA engines are otherwise idle here, and this
                # removes the xsum -> v dependency from batch 1's critical
                # path (costs one extra HBM read of batch 1).
                xr1 = x_d[1].rearrange(
                    "(p st j) d -> st p (j d)", p=P, st=NST, j=SJ
                )
                xsum1_ps = psum.tile([1, 2 * D], f32, name="xsum1_ps", bufs=1)
                for st in range(NST):
                    xs_t = xpool.tile([P, SJ * D], f32, name="xs_t", bufs=3)
                    nc.sync.dma_start(out=xs_t, in_=xr1[st])
                    for j2 in range(nmm):
                        nc.tensor.matmul(
                            xsum1_ps, ones_sb, xs_t[:, j2 * 512 : (j2 + 1) * 512],
                            start=(st == 0 and j2 == 0),
                            stop=(st == NST - 1 and j2 == nmm - 1),
                        )
                xsum1_half = small.tile([1, D], f32, name="xsum_half")
                nc.vector.tensor_copy(out=xsum1_half, in_=xsum1_ps[0:1, 0:D])
                nc.vector.tensor_add(
                    out=xsum1_half, in0=xsum1_half, in1=xsum1_ps[0:1, D : 2 * D]
                )
                prestream_v[0] = make_v(xsum1_half)

            # S2: s[p,t] = x_blk[p,:] . v  via fused multiply+row-reduce
            # (custom DVE op: out = (in0*1+0)*v_rep, accum_out = rowsum)
            s_sb = small.tile([P, NT], f32, name="s_sb")
            warm_ps = psum.tile([1, D], f32, name="warm_ps", bufs=1)
            for st in range(NST):
                for j in range(SJ):
                    t = st * SJ + j
                    # in place: the tile becomes prod = x*v; agg is
                    # recovered as (sum_n w*prod)/v at the end. Frees no
                    # extra SBUF and lets the next batch's DMA chase the
                    # score sweep (x's last reader would otherwise be S3).
                    blk = xt[st][:, j * D : (j + 1) * D]
                    nc.vector.affine_mul_reduce(
                        out=blk,
                        accum_out=s_sb[:, t : t + 1],
                        in0=blk,
                        in1=v_rep,
                        scale=1.0,
                        bias=0.0,
                    )
                    if b > 0 and t % 6 == 5:
                        # keep-warm trickle: PE re-throttles to 1.2 GHz after
                        # ~3.4us idle; a junk matmul every ~6 score blocks
                        # (input-dependent on the score just computed, so it
                        # interleaves in time) keeps it at 2.4 GHz. Batch 0's
                        # score window needs none - the batch-1 pre-stream
                        # matmuls keep the PE busy there anyway.
                        nc.tensor.matmul(warm_ps, ones_sb, blk, start=True, stop=True)

            # softmax over all 16384 scores of the batch
            m_p = small.tile([P, 1], f32, name="m_p")
            nc.vector.tensor_reduce(m_p, s_sb, axis=mybir.AxisListType.X, op=maxop)
            m_rep = small.tile([P, 1], f32, name="m_rep")
            nc.gpsimd.partition_all_reduce(
                m_rep, m_p, channels=P, reduce_op=bass_isa.ReduceOp.max
            )
            m_neg = small.tile([P, 1], f32, name="m_neg")
            nc.vector.tensor_scalar_mul(m_neg, m_rep, -1.0)
            e_sb = small.tile([P, NT], f32, name="e_sb")
            l_p = small.tile([P, 1], f32, name="l_p")
            nc.scalar.activation(
                out=e_sb,
                in_=s_sb,
                func=mybir.ActivationFunctionType.Exp,
                bias=m_neg,
                scale=1.0,
                accum_out=l_p,
            )
            l_rep = small.tile([P, 1], f32, name="l_rep")
            nc.gpsimd.partition_all_reduce(
                l_rep, l_p, channels=P, reduce_op=bass_isa.ReduceOp.add
            )
            linv = small.tile([P, 1], f32, name="linv")
            nc.vector.reciprocal(linv, l_rep)
            w_sb = small.tile([P, NT], f32, name="w_sb")
            nc.vector.tensor_scalar_mul(w_sb, e_sb, linv)
            if b == 0:
                nc.vector.tensor_copy(out=avgw_sb, in_=w_sb)
            else:
                nc.vector.tensor_add(out=avgw_sb, in0=avgw_sb, in1=w_sb)

            # S3: agg[1,d] = sum_t w[:,t].T @ x_blk, split across engines:
            # supertiles 0..7 as PE matmuls (w stationary), supertiles 8..15
            # as a DVE chain acc += x_blk * w[:,t] (per-partition scalar),
            # partition-reduced on PE at the end.
            agg_ps = psum.tile([1, D], f32, name="agg_ps", bufs=1)
            accA = small.tile([P, D], f32, name="accA", bufs=1)
            accB = small.tile([P, D], f32, name="accB", bufs=1)
            nc.vector.memset(accA, 0.0)
            nc.vector.memset(accB, 0.0)
            # batch 0 biases S3 toward PE so the DVE frees up sooner for
            # batch 1's scores (already runnable at that point); batch 1 is
            # terminal, so it balances the two engines evenly.
            s3_pe = 12 if (b == 0 and BPC > 1) else NST // 2
            for st in range(s3_pe):
                for j in range(SJ):
                    t = st * SJ + j
                    nc.tensor.matmul(
                        agg_ps,
                        w_sb[:, t : t + 1],
                        xt[st][:, j * D : (j + 1) * D],
                        start=(t == 0),
                        stop=False,
                    )
            # two independent in-place chains so consecutive DVE ops never
            # wait on each other
            for st in range(s3_pe, NST):
                for j in range(SJ):
                    t = st * SJ + j
                    acc = accA if j % 2 == 0 else accB
                    nc.vector.scalar_tensor_tensor(
                        out=acc,
                        in0=xt[st][:, j * D : (j + 1) * D],
                        scalar=w_sb[:, t : t + 1],
                        in1=acc,
                        op0=mult,
                        op1=addop,
                    )
            # fold the DVE accumulators into agg_ps (partition reduce)
            nc.tensor.matmul(agg_ps, ones_sb, accA, start=False, stop=False)
            nc.tensor.matmul(agg_ps, ones_sb, accB, start=False, stop=True)
            vinv = small.tile([1, D], f32, name="vinv")
            nc.vector.reciprocal(out=vinv, in_=v_rep[0:1, :])
            agg_sb = small.tile([1, D], f32, name="agg_sb")
            nc.vector.tensor_mul(out=agg_sb, in0=agg_ps, in1=vinv)
            nc.sync.dma_start(out=agg_d[b : b + 1, :], in_=agg_sb)

        nc.sync.dma_start(out=wsum_d.rearrange("(p t) -> p t", p=P), in_=avgw_sb)

    nc.compile()
    return nc


def _get_nc():
    if "nc" not in _compiled:
        _compiled["nc"] = _build_nc()
    return _compiled["nc"]


def _host_prep(inputs):
    x = np.ascontiguousarray(np.asarray(inputs["x"], dtype=np.float32))
    Wq = np.asarray(inputs["Wq"], dtype=np.float64)
    Wk = np.asarray(inputs["Wk"], dtype=np.float64)
    bk = np.asarray(inputs["bk"], dtype=np.float64)
    # M = Wq.T @ Wk ; device needs MT[c,i,d] = scale*M[d, 128c+i] = scale*(M.T)[128c+i, d]
    mt = (SCALE * (Wk.T @ Wq)).reshape(2, P, D).astype(np.float32)
    us = (SCALE * N * (Wq.T @ bk)).reshape(1, D).astype(np.float32)
    return x, np.ascontiguousarray(mt), np.ascontiguousarray(us)


def _run(inputs, **spmd_kwargs):
    from concourse.bass_utils import run_bass_kernel_spmd

    x, mt, us = _host_prep(inputs)
    nc = _get_nc()
    xs = x.reshape(NCORES, BPC, N, D)
    in_maps = [{"x_sh": xs[c], "mt": mt, "us": us} for c in range(NCORES)]
    res = run_bass_kernel_spmd(nc, in_maps, core_ids=list(range(NCORES)), **spmd_kwargs)
    agg = np.concatenate([res.results[c]["agg"] for c in range(NCORES)], axis=0)
    wsum = np.zeros(N, dtype=np.float64)
    for c in range(NCORES):
        wsum += res.results[c]["wsum"].astype(np.float64)
    avgw = (wsum / B).astype(np.float32)
    return (agg.astype(np.float32), avgw), res


def kernel(**inputs):
    out, _ = _run(inputs)
    return out
